# revision 19
# baseline (speedup 1.0000x reference)
"""Batch-softmax attention kernel for Trainium2 (8 NeuronCores), v7.

Problem: out[b,h,i,v] = sum_j softmax_over_b(QK^T/sqrt(H))[b,h,i,j] * V[b,h,j,v]
with B=4, H=8, S=2048, D=64.  Softmax is over the BATCH axis (dim=0).

Sharding: one head per NeuronCore (H=8 across 8 cores); batch softmax is
purely local.

Dataflow ("difference softmax"):
  W_b = E'_b * r',  E'_0 = 1,  E'_b = exp(scale*(s_b - s_0)) for b=1..3,
  r' = 1/(1 + E'_1 + E'_2 + E'_3).
  - PE: d_b = s_b - s_0 directly (stationary [k_b ; -k_0], moving [q_b ; q_0],
    full-128 contraction).  192 QK matmuls + 256 col-paired WV matmuls.
  - ACT: exp of 3 diffs per j-tile in one instr (PSUM [128,1536] -> bf16).
  - DVE: T = E'2+E'3, r' = ADD_RECIP_1P (recip(1+in0+in1)), all 3 W planes in
    one broadcast mul.  DVE is the busiest engine (~110us busy, ~97% occupied
    in its window) and paces the whole kernel; its three ops all run at the
    2-read-port/write-port element-rate floor, so further gains require a
    different softmax algebra, not scheduling.

Pipeline structure (v4->v7):
  - Software pipeline with adaptive depth: iteration g emits QK+exp(g), the
    DVE chain for g-1, and WV for g-2 (immediate DVE emission for the first
    ramp groups).  The DVE never waits on exps; the PE always has QK work
    while the DVE computes weights; deep tile pools (E x4, W x3) absorb
    chunk-boundary hiccups that otherwise idle the PE >3.4us and trigger a
    HAM re-throttle cascade (PE drops to 1.2 GHz).
  - Inputs land as 6 merged one-descriptor-per-partition DMAs, split across
    both HWDGE rings and ordered so the first K/Q tiles arrive first (K00 on
    sync || Q00 on scalar, then KA1, VT, KA2, QRT).
  - ~10 warmup matmuls on a zeroed scratch tile run during the DMA wait so
    the HAM clock gate reaches 2.4 GHz before real work; they dump into
    chunk-0's output PSUM tile, which the first real WV resets (start=True).
  - Tail: the last chunk's two PSUM evacuation copies run on ACT || DVE.

Tried and REVERTED (kept here so the next session doesn't re-try them):
  - W3 multiply on GPSIMD: the shared GPSIMD/DVE SBUF port nearly TRIPLED
    the DVE ADD cost (1223ns -> 2912ns) and GPSIMD's 2-input tensor op floor
    is ~3.6 cyc/el (6.1us per 4-jt group); net -40% overall.
  - Bulk input DMAs via GPSIMD SWDGE: slower than HWDGE, net loss.
  - Merged 128-partition output DMAs ([2,64,512] dram AP): descriptor
    lowering serialized the drain; 4x 64-partition DMAs are faster.
"""

import math
import os
import sys

import numpy as np

sys.path.insert(0, "/opt/trn_rl_repo")
os.environ.setdefault("MYCRO_LOCAL_CACHE", "1")

B, H, S, D = 4, 8, 2048, 64
N_CORES = 8
SCALE = 1.0 / math.sqrt(H)  # NOTE: reference scales by sqrt(num_heads)

IC = 4          # i-chunks of 512 columns
ICW = S // IC   # 512
JT = S // 128   # 16 j-tiles of 128 rows
N_WARM = int(os.environ.get("K_WARM", "8"))

_CACHED_NC = None
_ADD_RECIP_1P = None


def _register_add_recip_1p():
    """Custom DVE op: out = recip_approx(1 + in0 + in1), 1 Newton step."""
    global _ADD_RECIP_1P
    if _ADD_RECIP_1P is not None:
        return _ADD_RECIP_1P
    import numpy as np_
    import concourse.dve_ops as dvo
    from concourse.dve_spec import AluOp, Bin, C0, C1, One, Spec, Src0, Src1, lower
    from concourse.dve_uop import DveOpSpec

    _x = Bin(AluOp.ADD, Bin(AluOp.ADD, Src0, Src1), One)
    _nx = Bin(AluOp.BITWISE_NOT, _x, _x)
    _y0 = _nx * C0
    _body = _y0 * (C1 - _x * _y0)

    def _ref(in0, in1, s0, s1, imm2):
        x = (in0 + in1 + np_.float32(1.0)).astype(np_.float32)
        nx = (~x.view(np_.int32)).view(np_.float32)
        y0 = nx * np_.float32(s0)
        return y0 * (np_.float32(s1) - x * y0)

    name = "ADD_RECIP_1P_ANT"
    op = dvo.DveOp(name, Spec(body=_body, reference=_ref), subdim=False,
                   uops_sha={})
    dvo.OPS.append(op)
    dvo.CUSTOM_DVE_SPECS[name] = op.spec
    dvo._SUB_OPCODE_FOR_NAME[name] = dvo._CUSTOM_DVE_ROW_BASE + len(dvo.OPS) - 1
    assert dvo._SUB_OPCODE_FOR_NAME[name] < 0x20
    shas = {}
    for ver in ("v3", "v4"):
        s = DveOpSpec(name=name, opcode=dvo.get_dve_sub_opcode(name),
                      uops=lower(op.spec, ver=ver), rd1_en=True)
        shas[ver] = s.sha(ver)
    object.__setattr__(op, "uops_sha", shas)
    _ADD_RECIP_1P = op
    return op


def _build_nc():
    from concourse import bacc, tile
    from concourse.bass import mybir
    from concourse.dve_ops import RECIP_APPROX_FAST_CONSTS

    add_recip_1p = _register_add_recip_1p()

    f32 = mybir.dt.float32
    f16 = mybir.dt.float16
    bf16 = mybir.dt.bfloat16
    Exp = mybir.ActivationFunctionType.Exp
    rc = RECIP_APPROX_FAST_CONSTS

    nc = bacc.Bacc("TRN2", target_bir_lowering=False, debug=False)

    # merged per-core inputs (one head each); kd rows 64:128 hold -k0^T so the
    # matmul computes s_b - s_0 with full-128 contraction.
    # KQ0: K j-tiles 0-1 (3 diff-planes x 256) + chunk-0 Q (3 x 512)
    k00_in = nc.dram_tensor("k00", [128, 3 * 256], f16, kind="ExternalInput").ap()         # K jt 0-1
    q00_in = nc.dram_tensor("q00", [128, 3 * 512], f16, kind="ExternalInput").ap()         # Q chunk 0
    ka1_in = nc.dram_tensor("ka1", [128, 3 * 512], f16, kind="ExternalInput").ap()     # K jt 2-5
    ka2_in = nc.dram_tensor("ka2", [128, 3 * 1280], f16, kind="ExternalInput").ap()    # K jt 6-15
    qr_in = nc.dram_tensor("qr", [128, 3 * 1536], f16, kind="ExternalInput").ap()      # Q chunks 1-3
    vt_in = nc.dram_tensor("vt", [128, B * JT * D], bf16, kind="ExternalInput").ap()   # all V
    out_d = nc.dram_tensor("out", [B, D, S], f32, kind="ExternalOutput").ap()

    with tile.TileContext(nc) as tc:
        with (
            tc.tile_pool(name="wts", bufs=1) as wpool,
            tc.tile_pool(name="ep", bufs=4) as epool,
            tc.tile_pool(name="tp", bufs=3) as tpool,
            tc.tile_pool(name="rp", bufs=4) as rpool,
            tc.tile_pool(name="wp", bufs=3) as wpool2,
            tc.tile_pool(name="osb", bufs=2) as opool,
            tc.tile_pool(name="ps", bufs=2, space="PSUM") as psp,
            tc.tile_pool(name="po", bufs=1, space="PSUM") as pop,
        ):
            SCR_W = wpool.tile([128, 128], f16, tag="scrw")
            SCR_M = wpool.tile([128, ICW], f16, tag="scrm")
            K00 = wpool.tile([128, 3 * 256], f16, tag="k00")
            Q00 = wpool.tile([128, 3 * 512], f16, tag="q00")
            KA1 = wpool.tile([128, 3 * 512], f16, tag="ka1")
            KA2 = wpool.tile([128, 3 * 1280], f16, tag="ka2")
            QRT = wpool.tile([128, 3 * 1536], f16, tag="qrt")
            VT = wpool.tile([128, B * JT * D], bf16, tag="vt")

            nc.gpsimd.memset(SCR_W[:], 0.0)
            nc.gpsimd.memset(SCR_M[:], 0.0)
            # ordered input DMAs, split across both HWDGE rings so the
            # first K and Q tiles land concurrently and as early as possible
            nc.sync.dma_start(out=K00[:], in_=k00_in)
            nc.scalar.dma_start(out=Q00[:], in_=q00_in)
            nc.sync.dma_start(out=KA1[:], in_=ka1_in)
            nc.scalar.dma_start(out=VT[:], in_=vt_in)
            nc.sync.dma_start(out=KA2[:], in_=ka2_in)
            nc.sync.dma_start(out=QRT[:], in_=qr_in)

            # warmup matmuls: keep PE busy during the head-DMA wait so the
            # HAM clock-gate un-throttles before real QK work arrives.  They
            # write chunk-0's output PSUM tile, which the first real WV
            # accumulation resets via start=True.
            out_tiles = {0: (
                pop.tile([128, ICW], f32, tag="o01", name="o01_0"),
                pop.tile([128, ICW], f32, tag="o23", name="o23_0"))}
            for _ in range(N_WARM):
                nc.tensor.matmul(out_tiles[0][0][:], SCR_W[:], SCR_M[:],
                                 start=True, stop=True, skip_group_check=True)

            def k_slice(bi, jt):
                if jt < 2:
                    return K00[:, bi * 256 + jt * 128:bi * 256 + (jt + 1) * 128]
                if jt < 6:
                    return KA1[:, bi * 512 + (jt - 2) * 128:bi * 512 + (jt - 1) * 128]
                return KA2[:, bi * 1280 + (jt - 6) * 128:bi * 1280 + (jt - 5) * 128]

            def q_slice(bi, c):
                if c == 0:
                    return Q00[:, bi * 512:(bi + 1) * 512]
                return QRT[:, bi * 1536 + (c - 1) * 512:bi * 1536 + c * 512]

            def v_slice(b, jt):
                base = b * (JT * D) + jt * D
                return VT[:, base:base + D]

            # j-group schedule: small groups at the pipeline ramp (first
            # chunk) and drain (last chunk) shorten the serial critical path
            def schedule(c):
                if c == 0:
                    return [1, 1, 2, 2, 2, 2, 2, 4]
                if c == IC - 1:
                    return [4, 4, 4, 2, 1, 1]
                return [4, 4, 4, 4]

            groups = []  # (c, j0, JG, last_of_chunk)
            for c in range(IC):
                j0 = 0
                sch = schedule(c)
                for gi, JG in enumerate(sch):
                    groups.append((c, j0, JG, gi == len(sch) - 1))
                    j0 += JG

            def emit_qk_exp(c, j0, JG):
                E = epool.tile([128, JG, 3, ICW], bf16, tag="E",
                               name=f"E_{c}_{j0}")
                for u in range(JG):
                    jt = j0 + u
                    SP = psp.tile([128, 3, ICW], f32, tag="sp",
                                  name=f"sp_{c}_{jt}")
                    for bi in range(3):
                        nc.tensor.matmul(
                            SP[:, bi, :], k_slice(bi, jt), q_slice(bi, c),
                            start=True, stop=True)
                    nc.scalar.activation(E[:, u], SP[:], Exp, scale=SCALE)
                return E

            def emit_dve(c, j0, JG, E):
                T = tpool.tile([128, JG, ICW], bf16, tag="T", name=f"T_{c}_{j0}")
                nc.vector.tensor_add(T[:], E[:, :, 1, :].opt(),
                                     E[:, :, 2, :].opt())
                R = rpool.tile([128, JG, ICW], bf16, tag="R", name=f"R_{c}_{j0}")
                nc.vector._custom_dve(
                    add_recip_1p, out=R[:], in0=E[:, :, 0, :].opt(),
                    in1=T[:], s0=rc["s0"], s1=rc["s1"])
                W = wpool2.tile([128, JG, 3, ICW], bf16, tag="W", name=f"W_{c}_{j0}")
                # all three W's in ONE 2x op: contiguous dst, r broadcast over
                # the b' axis (0-stride)
                r3b = R[:].unsqueeze(2).broadcast_to([128, JG, 3, ICW])
                nc.vector.tensor_mul(W[:], E[:], r3b)
                return R, W

            def emit_wv(c, j0, JG, R, W):
                if c not in out_tiles:
                    out_tiles[c] = (
                        pop.tile([128, ICW], f32, tag="o01", name=f"o01_{c}"),
                        pop.tile([128, ICW], f32, tag="o23", name=f"o23_{c}"))
                out01, out23 = out_tiles[c]
                for u in range(JG):
                    jt = j0 + u
                    for b, (po_t, base) in enumerate(
                        [(out01, 0), (out01, 64), (out23, 0), (out23, 64)]
                    ):
                        rhs = R[:, u, :] if b == 0 else W[:, u, b - 1, :]
                        nc.tensor.matmul(
                            po_t[base:base + 64, :], v_slice(b, jt), rhs,
                            start=(jt == 0), stop=(jt == JT - 1),
                            tile_position=(0, base), skip_group_check=True)

            def finalize(c):
                out01, out23 = out_tiles[c]
                isl = slice(c * ICW, (c + 1) * ICW)
                OSB01 = opool.tile([128, ICW], f32, tag="osb01", name=f"ob1_{c}")
                OSB23 = opool.tile([128, ICW], f32, tag="osb23", name=f"ob2_{c}")
                last = c == IC - 1
                eng23 = nc.scalar if last else nc.sync
                nc.scalar.copy(OSB01[:], out01[:])
                nc.sync.dma_start(out=out_d[0, :, isl], in_=OSB01[0:64, :])
                nc.sync.dma_start(out=out_d[1, :, isl], in_=OSB01[64:128, :])
                if last:
                    # the Vector engine is idle at the drain: run the second
                    # copy there so both PSUM evacuations overlap
                    nc.vector.tensor_copy(OSB23[:], out23[:])
                else:
                    nc.scalar.copy(OSB23[:], out23[:])
                eng23.dma_start(out=out_d[2, :, isl], in_=OSB23[0:64, :])
                eng23.dma_start(out=out_d[3, :, isl], in_=OSB23[64:128, :])

            # Software pipeline with adaptive depth: the first groups emit
            # their DVE chain immediately (shortest ramp to get the DVE —
            # the pacing engine — started), then the pipeline settles into
            # the steady QK/exp(g) | DVE(g-1) | WV(g-2) pattern.
            dve_q = []  # groups awaiting DVE-chain emission
            wv_q = []   # groups awaiting WV emission
            for i, (c, j0, JG, is_last) in enumerate(groups):
                E = emit_qk_exp(c, j0, JG)
                dve_q.append((c, j0, JG, is_last, E))
                thr_d = 0 if i < 3 else 1
                if len(dve_q) > thr_d:
                    dc, dj0, dJG, dlast, dE = dve_q.pop(0)
                    R, W = emit_dve(dc, dj0, dJG, dE)
                    wv_q.append((dc, dj0, dJG, dlast, R, W))
                if len(wv_q) > 1:
                    wc, wj0, wJG, wlast, wR, wW = wv_q.pop(0)
                    emit_wv(wc, wj0, wJG, wR, wW)
                    if wlast:
                        finalize(wc)

            # drain
            for (dc, dj0, dJG, dlast, dE) in dve_q:
                R, W = emit_dve(dc, dj0, dJG, dE)
                wv_q.append((dc, dj0, dJG, dlast, R, W))
            for (wc, wj0, wJG, wlast, wR, wW) in wv_q:
                emit_wv(wc, wj0, wJG, wR, wW)
                if wlast:
                    finalize(wc)

    nc.compile()
    return nc


def _get_nc():
    global _CACHED_NC
    if _CACHED_NC is None:
        _CACHED_NC = _build_nc()
    return _CACHED_NC


def _make_in_maps(query, key, value):
    import ml_dtypes
    bf16 = ml_dtypes.bfloat16
    q16 = query.astype(np.float16)
    k16 = key.astype(np.float16)
    vbf = value.astype(bf16)
    in_maps = []
    for h in range(H):
        kT = [np.ascontiguousarray(k16[b, h].T) for b in range(B)]  # [64, S]
        qT = [np.ascontiguousarray(q16[b, h].T) for b in range(B)]
        nk0 = -kT[0]
        # kd[bi] = [k_{bi+1} ; -k_0]: [128, S];  qd[bi] = [q_{bi+1} ; q_0]
        kd = [np.concatenate([kT[b], nk0], axis=0) for b in (1, 2, 3)]
        qd = [np.concatenate([qT[b], qT[0]], axis=0) for b in (1, 2, 3)]
        k00 = np.concatenate([x[:, :256] for x in kd], axis=1)
        q00 = np.concatenate([x[:, :512] for x in qd], axis=1)
        ka1 = np.concatenate([x[:, 256:768] for x in kd], axis=1)
        ka2 = np.concatenate([x[:, 768:] for x in kd], axis=1)
        qr = np.concatenate([x[:, 512:] for x in qd], axis=1)
        # V: [128 j-in-tile, B * JT * D]
        vv = np.stack([vbf[b, h].reshape(JT, 128, D).transpose(1, 0, 2)
                       for b in range(B)], axis=1)  # [128, B, JT, D]
        im = {
            "k00": np.ascontiguousarray(k00),
            "q00": np.ascontiguousarray(q00),
            "ka1": np.ascontiguousarray(ka1),
            "ka2": np.ascontiguousarray(ka2),
            "qr": np.ascontiguousarray(qr),
            "vt": np.ascontiguousarray(vv.reshape(128, B * JT * D)),
        }
        in_maps.append(im)
    return in_maps


def _assemble(results):
    out = np.empty((B, H, S, D), np.float32)
    for h in range(H):
        out[:, h] = results[h]["out"].transpose(0, 2, 1)  # [B,D,S] -> [B,S,D]
    return out


def _install_profile_hook():
    """Provide antenv.axon_hooks with a ctypes NTFF profile hook so that
    run_bass_kernel_spmd(trace=True) works under axon in this container."""
    import contextlib
    import ctypes
    import types

    try:
        from antenv.axon_hooks import get_axon_ntff_profile_hook  # noqa: F401
        return  # already present
    except ImportError:
        pass

    so_path = "/opt/axon/libaxon_pjrt.so"
    lib = ctypes.CDLL(so_path)
    if not hasattr(lib, "axon_start_nrt_profile"):
        return
    lib.axon_start_nrt_profile.argtypes = [
        ctypes.POINTER(ctypes.c_int64), ctypes.c_size_t]
    lib.axon_start_nrt_profile.restype = ctypes.c_int64
    lib.axon_stop_nrt_profile.argtypes = [ctypes.c_char_p]
    lib.axon_stop_nrt_profile.restype = ctypes.c_int64

    @contextlib.contextmanager
    def _hook(output_dir, device_ids):
        import jax
        jax.devices()
        if device_ids:
            ids = (ctypes.c_int64 * len(device_ids))(*device_ids)
            rc = lib.axon_start_nrt_profile(ids, len(device_ids))
        else:
            rc = lib.axon_start_nrt_profile(None, 0)
        if rc != 0:
            raise RuntimeError(f"axon_start_nrt_profile rc={rc}")
        try:
            yield
        finally:
            n = lib.axon_stop_nrt_profile(str(output_dir).encode())
            print(f"ntff profile: {n} file(s) written to {output_dir}")

    mod = types.ModuleType("antenv.axon_hooks")
    mod.get_axon_ntff_profile_hook = lambda: _hook
    mod.set_axon_ntff_profile_hook = lambda h: None
    sys.modules["antenv.axon_hooks"] = mod


def run(query, key, value, trace=False):
    """Run the distributed kernel; returns (output, exec_time_ns or None)."""
    from concourse.bass_utils import run_bass_kernel_spmd

    if trace:
        _install_profile_hook()
    nc = _get_nc()
    in_maps = _make_in_maps(query, key, value)
    res = run_bass_kernel_spmd(nc, in_maps, core_ids=list(range(N_CORES)),
                               trace=trace)
    return _assemble(res.results), res.exec_time_ns


def kernel(query, key, value):
    out, _ = run(query, key, value, trace=False)
    return out


# revision 20
# speedup vs baseline: 1.0297x; 1.0297x over previous
"""Batch-softmax attention kernel for Trainium2 (8 NeuronCores), v7.

Problem: out[b,h,i,v] = sum_j softmax_over_b(QK^T/sqrt(H))[b,h,i,j] * V[b,h,j,v]
with B=4, H=8, S=2048, D=64.  Softmax is over the BATCH axis (dim=0).

Sharding: one head per NeuronCore (H=8 across 8 cores); batch softmax is
purely local.

Dataflow ("difference softmax"):
  W_b = E'_b * r',  E'_0 = 1,  E'_b = exp(scale*(s_b - s_0)) for b=1..3,
  r' = 1/(1 + E'_1 + E'_2 + E'_3).
  - PE: d_b = s_b - s_0 directly (stationary [k_b ; -k_0], moving [q_b ; q_0],
    full-128 contraction).  192 QK matmuls + 256 col-paired WV matmuls.
  - ACT: exp of 3 diffs per j-tile in one instr (PSUM [128,1536] -> bf16).
  - DVE: T = E'2+E'3, r' = ADD_RECIP_1P (recip(1+in0+in1)), all 3 W planes in
    one broadcast mul.  DVE is the busiest engine (~110us busy, ~97% occupied
    in its window) and paces the whole kernel; its three ops all run at the
    2-read-port/write-port element-rate floor, so further gains require a
    different softmax algebra, not scheduling.

Pipeline structure (v4->v7):
  - Software pipeline with adaptive depth: iteration g emits QK+exp(g), the
    DVE chain for g-1, and WV for g-2 (immediate DVE emission for the first
    ramp groups).  The DVE never waits on exps; the PE always has QK work
    while the DVE computes weights; deep tile pools (E x4, W x3) absorb
    chunk-boundary hiccups that otherwise idle the PE >3.4us and trigger a
    HAM re-throttle cascade (PE drops to 1.2 GHz).
  - Inputs land as 6 merged one-descriptor-per-partition DMAs, split across
    both HWDGE rings and ordered so the first K/Q tiles arrive first (K00 on
    sync || Q00 on scalar, then KA1, VT, KA2, QRT).
  - ~10 warmup matmuls on a zeroed scratch tile run during the DMA wait so
    the HAM clock gate reaches 2.4 GHz before real work; they dump into
    chunk-0's output PSUM tile, which the first real WV resets (start=True).
  - Tail: the last chunk's two PSUM evacuation copies run on ACT || DVE.

Tried and REVERTED (kept here so the next session doesn't re-try them):
  - W3 multiply on GPSIMD: the shared GPSIMD/DVE SBUF port nearly TRIPLED
    the DVE ADD cost (1223ns -> 2912ns) and GPSIMD's 2-input tensor op floor
    is ~3.6 cyc/el (6.1us per 4-jt group); net -40% overall.
  - Bulk input DMAs via GPSIMD SWDGE: slower than HWDGE, net loss.
  - Merged 128-partition output DMAs ([2,64,512] dram AP): descriptor
    lowering serialized the drain; 4x 64-partition DMAs are faster.
"""

import math
import os
import sys

import numpy as np

sys.path.insert(0, "/opt/trn_rl_repo")
os.environ.setdefault("MYCRO_LOCAL_CACHE", "1")

B, H, S, D = 4, 8, 2048, 64
N_CORES = 8
SCALE = 1.0 / math.sqrt(H)  # NOTE: reference scales by sqrt(num_heads)

IC = 4          # i-chunks of 512 columns
ICW = S // IC   # 512
JT = S // 128   # 16 j-tiles of 128 rows
N_WARM = int(os.environ.get("K_WARM", "8"))

_CACHED_NC = None
_ADD_RECIP_1P = None


def _register_add_recip_1p():
    """Custom DVE op: out = recip_approx(1 + in0 + in1), 1 Newton step."""
    global _ADD_RECIP_1P
    if _ADD_RECIP_1P is not None:
        return _ADD_RECIP_1P
    import numpy as np_
    import concourse.dve_ops as dvo
    from concourse.dve_spec import AluOp, Bin, C0, C1, One, Spec, Src0, Src1, lower
    from concourse.dve_uop import DveOpSpec

    _x = Bin(AluOp.ADD, Bin(AluOp.ADD, Src0, Src1), One)
    _nx = Bin(AluOp.BITWISE_NOT, _x, _x)
    _y0 = _nx * C0
    _body = _y0 * (C1 - _x * _y0)

    def _ref(in0, in1, s0, s1, imm2):
        x = (in0 + in1 + np_.float32(1.0)).astype(np_.float32)
        nx = (~x.view(np_.int32)).view(np_.float32)
        y0 = nx * np_.float32(s0)
        return y0 * (np_.float32(s1) - x * y0)

    name = "ADD_RECIP_1P_ANT"
    op = dvo.DveOp(name, Spec(body=_body, reference=_ref), subdim=False,
                   uops_sha={})
    dvo.OPS.append(op)
    dvo.CUSTOM_DVE_SPECS[name] = op.spec
    dvo._SUB_OPCODE_FOR_NAME[name] = dvo._CUSTOM_DVE_ROW_BASE + len(dvo.OPS) - 1
    assert dvo._SUB_OPCODE_FOR_NAME[name] < 0x20
    shas = {}
    for ver in ("v3", "v4"):
        s = DveOpSpec(name=name, opcode=dvo.get_dve_sub_opcode(name),
                      uops=lower(op.spec, ver=ver), rd1_en=True)
        shas[ver] = s.sha(ver)
    object.__setattr__(op, "uops_sha", shas)
    _ADD_RECIP_1P = op
    return op


def _build_nc():
    from concourse import bacc, tile
    from concourse.bass import mybir
    from concourse.dve_ops import RECIP_APPROX_FAST_CONSTS

    add_recip_1p = _register_add_recip_1p()

    f32 = mybir.dt.float32
    f16 = mybir.dt.float16
    bf16 = mybir.dt.bfloat16
    Exp = mybir.ActivationFunctionType.Exp
    rc = RECIP_APPROX_FAST_CONSTS

    nc = bacc.Bacc("TRN2", target_bir_lowering=False, debug=False)

    # merged per-core inputs (one head each); kd rows 64:128 hold -k0^T so the
    # matmul computes s_b - s_0 with full-128 contraction.
    # KQ0: K j-tiles 0-1 (3 diff-planes x 256) + chunk-0 Q (3 x 512)
    kq0_in = nc.dram_tensor("kq0", [128, 3 * 256 + 3 * 512], f16, kind="ExternalInput").ap()  # K jt 0-1 + Q chunk 0
    ka1_in = nc.dram_tensor("ka1", [128, 3 * 512], f16, kind="ExternalInput").ap()     # K jt 2-5
    ka2_in = nc.dram_tensor("ka2", [128, 3 * 1280], f16, kind="ExternalInput").ap()    # K jt 6-15
    qr_in = nc.dram_tensor("qr", [128, 3 * 1536], f16, kind="ExternalInput").ap()      # Q chunks 1-3
    vt_in = nc.dram_tensor("vt", [128, B * JT * D], bf16, kind="ExternalInput").ap()   # all V
    out_d = nc.dram_tensor("out", [B, D, S], f32, kind="ExternalOutput").ap()

    with tile.TileContext(nc) as tc:
        with (
            tc.tile_pool(name="wts", bufs=1) as wpool,
            tc.tile_pool(name="ep", bufs=4) as epool,
            tc.tile_pool(name="tp", bufs=3) as tpool,
            tc.tile_pool(name="rp", bufs=4) as rpool,
            tc.tile_pool(name="wp", bufs=3) as wpool2,
            tc.tile_pool(name="osb", bufs=2) as opool,
            tc.tile_pool(name="ps", bufs=2, space="PSUM") as psp,
            tc.tile_pool(name="po", bufs=1, space="PSUM") as pop,
        ):
            SCR_W = wpool.tile([128, 128], f16, tag="scrw")
            SCR_M = wpool.tile([128, ICW], f16, tag="scrm")
            KQ0 = wpool.tile([128, 3 * 256 + 3 * 512], f16, tag="kq0")
            KA1 = wpool.tile([128, 3 * 512], f16, tag="ka1")
            KA2 = wpool.tile([128, 3 * 1280], f16, tag="ka2")
            QRT = wpool.tile([128, 3 * 1536], f16, tag="qrt")
            VT = wpool.tile([128, B * JT * D], bf16, tag="vt")

            nc.gpsimd.memset(SCR_W[:], 0.0)
            nc.gpsimd.memset(SCR_M[:], 0.0)
            # ALL input DMAs ride the sync HWDGE ring, FIFO: the combined
            # first tile (K jt0-1 + Q chunk0, ONE descriptor per partition)
            # monopolizes all 16 SDMA engines and lands first; nothing is
            # issued on the scalar ring to round-robin against it
            nc.sync.dma_start(out=KQ0[:], in_=kq0_in)
            nc.sync.dma_start(out=KA1[:], in_=ka1_in)
            nc.sync.dma_start(out=VT[:], in_=vt_in)
            nc.sync.dma_start(out=KA2[:], in_=ka2_in)
            nc.sync.dma_start(out=QRT[:], in_=qr_in)

            # warmup matmuls: keep PE busy during the head-DMA wait so the
            # HAM clock-gate un-throttles before real QK work arrives.  They
            # write chunk-0's output PSUM tile, which the first real WV
            # accumulation resets via start=True.
            out_tiles = {0: (
                pop.tile([128, ICW], f32, tag="o01", name="o01_0"),
                pop.tile([128, ICW], f32, tag="o23", name="o23_0"))}
            for _ in range(N_WARM):
                nc.tensor.matmul(out_tiles[0][0][:], SCR_W[:], SCR_M[:],
                                 start=True, stop=True, skip_group_check=True)

            def k_slice(bi, jt):
                if jt < 2:
                    return KQ0[:, bi * 256 + jt * 128:bi * 256 + (jt + 1) * 128]
                if jt < 6:
                    return KA1[:, bi * 512 + (jt - 2) * 128:bi * 512 + (jt - 1) * 128]
                return KA2[:, bi * 1280 + (jt - 6) * 128:bi * 1280 + (jt - 5) * 128]

            def q_slice(bi, c):
                if c == 0:
                    return KQ0[:, 768 + bi * 512:768 + (bi + 1) * 512]
                return QRT[:, bi * 1536 + (c - 1) * 512:bi * 1536 + c * 512]

            def v_slice(b, jt):
                base = b * (JT * D) + jt * D
                return VT[:, base:base + D]

            # j-group schedule: small groups at the pipeline ramp (first
            # chunk) and drain (last chunk) shorten the serial critical path
            def schedule(c):
                if c == 0:
                    return [1, 1, 2, 2, 2, 2, 2, 4]
                if c == IC - 1:
                    return [4, 4, 4, 2, 1, 1]
                return [4, 4, 4, 4]

            groups = []  # (c, j0, JG, last_of_chunk)
            for c in range(IC):
                j0 = 0
                sch = schedule(c)
                for gi, JG in enumerate(sch):
                    groups.append((c, j0, JG, gi == len(sch) - 1))
                    j0 += JG

            def emit_qk_exp(c, j0, JG):
                E = epool.tile([128, JG, 3, ICW], bf16, tag="E",
                               name=f"E_{c}_{j0}")
                for u in range(JG):
                    jt = j0 + u
                    SP = psp.tile([128, 3, ICW], f32, tag="sp",
                                  name=f"sp_{c}_{jt}")
                    for bi in range(3):
                        nc.tensor.matmul(
                            SP[:, bi, :], k_slice(bi, jt), q_slice(bi, c),
                            start=True, stop=True)
                    nc.scalar.activation(E[:, u], SP[:], Exp, scale=SCALE)
                return E

            def emit_dve(c, j0, JG, E):
                T = tpool.tile([128, JG, ICW], bf16, tag="T", name=f"T_{c}_{j0}")
                nc.vector.tensor_add(T[:], E[:, :, 1, :].opt(),
                                     E[:, :, 2, :].opt())
                R = rpool.tile([128, JG, ICW], bf16, tag="R", name=f"R_{c}_{j0}")
                nc.vector._custom_dve(
                    add_recip_1p, out=R[:], in0=E[:, :, 0, :].opt(),
                    in1=T[:], s0=rc["s0"], s1=rc["s1"])
                W = wpool2.tile([128, JG, 3, ICW], bf16, tag="W", name=f"W_{c}_{j0}")
                # all three W's in ONE 2x op: contiguous dst, r broadcast over
                # the b' axis (0-stride)
                r3b = R[:].unsqueeze(2).broadcast_to([128, JG, 3, ICW])
                nc.vector.tensor_mul(W[:], E[:], r3b)
                return R, W

            def emit_wv(c, j0, JG, R, W):
                if c not in out_tiles:
                    out_tiles[c] = (
                        pop.tile([128, ICW], f32, tag="o01", name=f"o01_{c}"),
                        pop.tile([128, ICW], f32, tag="o23", name=f"o23_{c}"))
                out01, out23 = out_tiles[c]
                for u in range(JG):
                    jt = j0 + u
                    for b, (po_t, base) in enumerate(
                        [(out01, 0), (out01, 64), (out23, 0), (out23, 64)]
                    ):
                        rhs = R[:, u, :] if b == 0 else W[:, u, b - 1, :]
                        nc.tensor.matmul(
                            po_t[base:base + 64, :], v_slice(b, jt), rhs,
                            start=(jt == 0), stop=(jt == JT - 1),
                            tile_position=(0, base), skip_group_check=True)

            def finalize(c):
                out01, out23 = out_tiles[c]
                isl = slice(c * ICW, (c + 1) * ICW)
                OSB01 = opool.tile([128, ICW], f32, tag="osb01", name=f"ob1_{c}")
                OSB23 = opool.tile([128, ICW], f32, tag="osb23", name=f"ob2_{c}")
                last = c == IC - 1
                eng23 = nc.scalar if last else nc.sync
                nc.scalar.copy(OSB01[:], out01[:])
                nc.sync.dma_start(out=out_d[0, :, isl], in_=OSB01[0:64, :])
                nc.sync.dma_start(out=out_d[1, :, isl], in_=OSB01[64:128, :])
                if last:
                    # the Vector engine is idle at the drain: run the second
                    # copy there so both PSUM evacuations overlap
                    nc.vector.tensor_copy(OSB23[:], out23[:])
                else:
                    nc.scalar.copy(OSB23[:], out23[:])
                eng23.dma_start(out=out_d[2, :, isl], in_=OSB23[0:64, :])
                eng23.dma_start(out=out_d[3, :, isl], in_=OSB23[64:128, :])

            # Software pipeline with adaptive depth: the first groups emit
            # their DVE chain immediately (shortest ramp to get the DVE —
            # the pacing engine — started), then the pipeline settles into
            # the steady QK/exp(g) | DVE(g-1) | WV(g-2) pattern.
            dve_q = []  # groups awaiting DVE-chain emission
            wv_q = []   # groups awaiting WV emission
            for i, (c, j0, JG, is_last) in enumerate(groups):
                E = emit_qk_exp(c, j0, JG)
                dve_q.append((c, j0, JG, is_last, E))
                thr_d = 0 if i < 3 else 1
                if len(dve_q) > thr_d:
                    dc, dj0, dJG, dlast, dE = dve_q.pop(0)
                    R, W = emit_dve(dc, dj0, dJG, dE)
                    wv_q.append((dc, dj0, dJG, dlast, R, W))
                if len(wv_q) > 1:
                    wc, wj0, wJG, wlast, wR, wW = wv_q.pop(0)
                    emit_wv(wc, wj0, wJG, wR, wW)
                    if wlast:
                        finalize(wc)

            # drain
            for (dc, dj0, dJG, dlast, dE) in dve_q:
                R, W = emit_dve(dc, dj0, dJG, dE)
                wv_q.append((dc, dj0, dJG, dlast, R, W))
            for (wc, wj0, wJG, wlast, wR, wW) in wv_q:
                emit_wv(wc, wj0, wJG, wR, wW)
                if wlast:
                    finalize(wc)

    nc.compile()
    return nc


def _get_nc():
    global _CACHED_NC
    if _CACHED_NC is None:
        _CACHED_NC = _build_nc()
    return _CACHED_NC


def _make_in_maps(query, key, value):
    import ml_dtypes
    bf16 = ml_dtypes.bfloat16
    q16 = query.astype(np.float16)
    k16 = key.astype(np.float16)
    vbf = value.astype(bf16)
    in_maps = []
    for h in range(H):
        kT = [np.ascontiguousarray(k16[b, h].T) for b in range(B)]  # [64, S]
        qT = [np.ascontiguousarray(q16[b, h].T) for b in range(B)]
        nk0 = -kT[0]
        # kd[bi] = [k_{bi+1} ; -k_0]: [128, S];  qd[bi] = [q_{bi+1} ; q_0]
        kd = [np.concatenate([kT[b], nk0], axis=0) for b in (1, 2, 3)]
        qd = [np.concatenate([qT[b], qT[0]], axis=0) for b in (1, 2, 3)]
        kq0 = np.concatenate([x[:, :256] for x in kd] +
                             [x[:, :512] for x in qd], axis=1)
        ka1 = np.concatenate([x[:, 256:768] for x in kd], axis=1)
        ka2 = np.concatenate([x[:, 768:] for x in kd], axis=1)
        qr = np.concatenate([x[:, 512:] for x in qd], axis=1)
        # V: [128 j-in-tile, B * JT * D]
        vv = np.stack([vbf[b, h].reshape(JT, 128, D).transpose(1, 0, 2)
                       for b in range(B)], axis=1)  # [128, B, JT, D]
        im = {
            "kq0": np.ascontiguousarray(kq0),
            "ka1": np.ascontiguousarray(ka1),
            "ka2": np.ascontiguousarray(ka2),
            "qr": np.ascontiguousarray(qr),
            "vt": np.ascontiguousarray(vv.reshape(128, B * JT * D)),
        }
        in_maps.append(im)
    return in_maps


def _assemble(results):
    out = np.empty((B, H, S, D), np.float32)
    for h in range(H):
        out[:, h] = results[h]["out"].transpose(0, 2, 1)  # [B,D,S] -> [B,S,D]
    return out


def _install_profile_hook():
    """Provide antenv.axon_hooks with a ctypes NTFF profile hook so that
    run_bass_kernel_spmd(trace=True) works under axon in this container."""
    import contextlib
    import ctypes
    import types

    try:
        from antenv.axon_hooks import get_axon_ntff_profile_hook  # noqa: F401
        return  # already present
    except ImportError:
        pass

    so_path = "/opt/axon/libaxon_pjrt.so"
    lib = ctypes.CDLL(so_path)
    if not hasattr(lib, "axon_start_nrt_profile"):
        return
    lib.axon_start_nrt_profile.argtypes = [
        ctypes.POINTER(ctypes.c_int64), ctypes.c_size_t]
    lib.axon_start_nrt_profile.restype = ctypes.c_int64
    lib.axon_stop_nrt_profile.argtypes = [ctypes.c_char_p]
    lib.axon_stop_nrt_profile.restype = ctypes.c_int64

    @contextlib.contextmanager
    def _hook(output_dir, device_ids):
        import jax
        jax.devices()
        if device_ids:
            ids = (ctypes.c_int64 * len(device_ids))(*device_ids)
            rc = lib.axon_start_nrt_profile(ids, len(device_ids))
        else:
            rc = lib.axon_start_nrt_profile(None, 0)
        if rc != 0:
            raise RuntimeError(f"axon_start_nrt_profile rc={rc}")
        try:
            yield
        finally:
            n = lib.axon_stop_nrt_profile(str(output_dir).encode())
            print(f"ntff profile: {n} file(s) written to {output_dir}")

    mod = types.ModuleType("antenv.axon_hooks")
    mod.get_axon_ntff_profile_hook = lambda: _hook
    mod.set_axon_ntff_profile_hook = lambda h: None
    sys.modules["antenv.axon_hooks"] = mod


def run(query, key, value, trace=False):
    """Run the distributed kernel; returns (output, exec_time_ns or None)."""
    from concourse.bass_utils import run_bass_kernel_spmd

    if trace:
        _install_profile_hook()
    nc = _get_nc()
    in_maps = _make_in_maps(query, key, value)
    res = run_bass_kernel_spmd(nc, in_maps, core_ids=list(range(N_CORES)),
                               trace=trace)
    return _assemble(res.results), res.exec_time_ns


def kernel(query, key, value):
    out, _ = run(query, key, value, trace=False)
    return out
